# revision 74
# baseline (speedup 1.0000x reference)
"""AttRNN Trainium2 kernel (8-core SPMD, full I/O).

Model: y = feat @ W.T + b where feat = [ctx, last] from a 512-step tanh RNN
with dot-product attention over time, on embedded tokens.

Strategy (identical program on all 8 cores, divergence only via input data):
  - Embedding gather + input projection (xin) + the sequential RNN are
    replicated on every core (the RNN is the serial critical path; B=64 is too
    small to shard without per-step collectives).
  - All matmuls run in fp16 (1 cycle/row on the PE vs 4 for fp32;
    validated absmax rel err ~1.5e-3 vs the fp32 reference).
  - The [32000, 1024] output projection W is sharded by vocab: core c gets
    rows [4000c, 4000(c+1)) and computes out[:, slice]; host concatenates.
  - RNN layout: hT ("transposed", [h-part, batch-free]) so each step is
    16 self-loading fp16 matmuls (stationary = W_hh.T blocks), one DVE add of
    the precomputed xinT per half, one ACT tanh per half -> next hT.
    No per-step transposes.
  - Weight/x transposes via PE transpose (identity matmul) + PSUM evacuation.
  - Attention is s-sharded: each core keeps its 64 steps of H in SBUF via a
    per-step DVE copy_predicated into slot (t mod 64), predicated on a
    per-core 0/1 mask input. H is pre-zeroed; softmax subtracts an |last|^2
    shift (identical on every core), so the never-written slot contributes
    exp(0 - shift) ~ 0. One 160KB AllReduce combines ctx partials + Z.
"""

import os
import sys

sys.path.insert(0, "/opt/trn_rl_repo")

import numpy as np

# ---------------------------------------------------------------- constants
B, S, E, H, V = 64, 512, 256, 512, 32000
NC = 8
JC = V // NC            # 4000 vocab rows per core
CH = 8                  # RNN steps per chunk
NCHUNK = S // CH
TOKCH = CH * B
GPC = TOKCH // 128
HSLOT = S // NC         # 64 steps of H kept per core

_CACHE = {}


def _build():
    import concourse.bass as bass
    import concourse.mybir as mybir
    from concourse import bacc
    from concourse.tile import TileContext

    dt = mybir.dt
    AF = mybir.ActivationFunctionType
    ALU = mybir.AluOpType
    f32, f16, i32 = dt.float32, dt.float16, dt.int32

    nc = bacc.Bacc("TRN2", target_bir_lowering=False, num_devices=NC)

    # ------------------------------------------------------------- I/O
    emb_d = nc.dram_tensor("emb", [V, E], f32, kind="ExternalInput")
    wih_d = nc.dram_tensor("wih", [H, E], f32, kind="ExternalInput")
    whh_d = nc.dram_tensor("whh", [H, H], f32, kind="ExternalInput")
    bias4_d = nc.dram_tensor("bias4", [128, 4], f32, kind="ExternalInput")
    wc_d = nc.dram_tensor("wc", [JC, 2 * H], f32, kind="ExternalInput")
    bcb_d = nc.dram_tensor("bcb", [B, JC], f32, kind="ExternalInput")
    xidx_d = nc.dram_tensor("xidx", [128, S // 2], i32, kind="ExternalInput")
    hmask_d = nc.dram_tensor("hmask", [1, S], dt.int16, kind="ExternalInput")
    eye_d = nc.dram_tensor("eye", [128, 128], f16, kind="ExternalInput")
    out_d = nc.dram_tensor("out", [B, JC], f32, kind="ExternalOutput")

    with TileContext(nc) as tc:
        with (
            tc.tile_pool(name="const", bufs=1) as constp,
            tc.tile_pool(name="wt", bufs=1) as wtp,
            tc.tile_pool(name="stage", bufs=2) as stgp,
            tc.tile_pool(name="gat", bufs=4) as gatp,
            tc.tile_pool(name="xt", bufs=3) as xtp,
            tc.tile_pool(name="xin", bufs=3) as xinp,
            tc.tile_pool(name="rnn", bufs=4) as rnnp,
            tc.tile_pool(name="att", bufs=1) as attp,
            tc.tile_pool(name="fin", bufs=2) as finp,
            tc.tile_pool(name="ps", bufs=2, space="PSUM") as psp,
            tc.tile_pool(name="dram", bufs=1, space="DRAM") as drp,
        ):
            # ------------------------------------------------ resident tiles
            xidx = constp.tile([128, S // 2], i32, name="xidx")
            nc.sync.dma_start(xidx[:], xidx_d[:])
            bias4 = constp.tile([128, 4], f32, name="bias4")
            nc.sync.dma_start(bias4[:], bias4_d[:])
            ones = constp.tile([128, 1], f16, name="ones")
            eye = constp.tile([128, 128], f16, name="eye")
            nc.sync.dma_start(eye[:], eye_d[:])
            hmk1 = constp.tile([1, S], dt.int16, name="hmk1")
            nc.sync.dma_start(hmk1[:], hmask_d[:])
            maskb = constp.tile([128, S], dt.int16, name="maskb")
            # H window, selected by predicated writes. Only slot 63 needs
            # pre-zeroing: every other slot is pred-written by some step on
            # every core; slot 63 stays unwritten on core 7 (s=511 is
            # 'last') and must score exp(0 - shift) ~ 0
            hbig = constp.tile([128, HSLOT * 4 * B], f16, name="hbig")
            nc.vector.memset(
                hbig[:, (HSLOT - 1) * 4 * B:HSLOT * 4 * B], 0.0
            )
            last_t = constp.tile([128, 4 * B], f16, name="last_t")

            def pe_transpose(dst, src, rr, evac):
                """dst[:, :rr] (SBUF fp16) = src[:rr, :128].T via PE."""
                pst = psp.tile(
                    [128, 128], f16, space="PSUM", name="pst", tag="pst"
                )
                nc.tensor.transpose(pst[:, :rr], src, eye[:rr, :rr])
                # evacuate on DVE: keeps the ACT queue clear for the RNN's
                # chain-critical tanh (ACT is strict FIFO)
                nc.vector.tensor_copy(dst, pst[:, :rr])

            # ----------------------------------------- weight prep (fp16 + T)
            def load_transposed(src_d, rows, cols, dst_fn, tagb):
                """src_d [rows, cols] f32 DRAM; for each 128-col block cb,
                dst_fn(cb)[:, r:r+rr] = src[r:r+rr, cb-block].T (fp16)."""
                for i, r in enumerate(range(0, rows, 128)):
                    rr = min(128, rows - r)
                    natf = stgp.tile(
                        [128, cols], f32, name="natf", tag=f"natf{tagb}"
                    )
                    nc.sync.dma_start(natf[:rr, :], src_d[r:r + rr, :])
                    nat = stgp.tile(
                        [128, cols], f16, name="nat", tag=f"nat{tagb}"
                    )
                    nc.gpsimd.tensor_copy(nat[:rr, :], natf[:rr, :])
                    for cb in range(cols // 128):
                        pe_transpose(
                            dst_fn(cb)[:, r:r + rr],
                            nat[:rr, cb * 128:(cb + 1) * 128],
                            rr,
                            (i + cb) % 2,
                        )

            wihT = [wtp.tile([128, H], f16, name=f"wihT{e}") for e in range(2)]
            load_transposed(wih_d, H, E, lambda cb: wihT[cb], "w")
            whhT = [wtp.tile([128, H], f16, name=f"whhT{k}") for k in range(4)]

            # Wc cast+transpose staged across RNN chunks
            wcT = [wtp.tile([128, JC], f16, name=f"wcT{k}") for k in range(8)]

            def wc_cast_slice(r):
                rows = min(128, JC - r * 128)
                if rows <= 0:
                    return
                natf = stgp.tile([128, 2 * H], f32, name="natfc", tag="natfc")
                nc.sync.dma_start(
                    natf[:rows, :], wc_d[r * 128:r * 128 + rows, :]
                )
                nat = stgp.tile([128, 2 * H], f16, name="natc", tag="natc")
                nc.gpsimd.tensor_copy(nat[:rows, :], natf[:rows, :])
                for cb in range(8):
                    pe_transpose(
                        wcT[cb][:, r * 128:r * 128 + rows],
                        nat[:rows, cb * 128:(cb + 1) * 128],
                        rows,
                        (r + cb) % 2,
                    )

            # ------------------------------------------------ gather + xin
            def emit_chunk(c):
                xT = [
                    xtp.tile([128, TOKCH], f16, name=f"xT{e}", tag=f"xT{e}")
                    for e in range(2)
                ]
                for g in range(GPC):
                    gg = c * GPC + g
                    xg = gatp.tile([128, E], f32, name="xg", tag="xg")
                    nc.gpsimd.indirect_dma_start(
                        out=xg[:],
                        out_offset=None,
                        in_=emb_d[:],
                        in_offset=bass.IndirectOffsetOnAxis(
                            ap=xidx[:, gg:gg + 1], axis=0
                        ),
                    )
                    x16 = gatp.tile([128, E], f16, name="x16", tag="x16")
                    nc.gpsimd.tensor_copy(x16[:], xg[:])
                    for e in range(2):
                        pe_transpose(
                            xT[e][:, g * 128:(g + 1) * 128],
                            x16[:, e * 128:(e + 1) * 128],
                            128,
                            (g + e) % 2,
                        )
                # xinT chunk [128, (t CH, m 4, b B)] fp16, bias folded in.
                # The 4 per-m matmul groups are emitted lazily (one every
                # two RNN steps) so their psum WAR + evacuation never pile
                # up at a chunk boundary in the PE FIFO.
                xin = xinp.tile([128, CH * 4 * B], f16, name="xin", tag="xin")
                xin4 = xin.rearrange("p (t m b) -> p t m b", t=CH, m=4, b=B)

                def emit_group(m):
                    ps = psp.tile(
                        [128, 512], f32, space="PSUM", name="psx", tag="psx"
                    )
                    for e in range(2):
                        nc.tensor.matmul(
                            ps[:],
                            lhsT=wihT[e][:, m * 128:(m + 1) * 128],
                            rhs=xT[e][:],
                            start=(e == 0),
                            stop=(e == 1),
                        )
                    nc.vector.tensor_scalar(
                        out=xin4[:, :, m, :],
                        in0=ps.rearrange("p (t b) -> p t b", t=CH, b=B),
                        scalar1=bias4[:, m:m + 1],
                        scalar2=None,
                        op0=ALU.add,
                    )
                return xin, emit_group

            # ------------------------------------------------ RNN
            # prefetch distance 2: chunk c+2's gather/transpose/xin work
            # is emitted during chunk c, so its matmuls never head-of-line
            # block the PE sequencer on not-yet-ready inputs
            x0, g0 = emit_chunk(0)
            for m in range(4):
                g0(m)
            # W_hh prep + mask/ones setup deferred here: first needed at RNN
            # step 1 / step 0's pred-copy, so they stay off the cold-start
            # critical path (chunk-0 gather -> xin -> step 0)
            load_transposed(whh_d, H, H, lambda cb: whhT[cb], "w")
            nc.gpsimd.partition_broadcast(maskb[:], hmk1[:])
            nc.vector.memset(ones[:], 1.0)
            x1, g1 = emit_chunk(1)
            for m in range(4):
                g1(m)
            xin_q = [x0, x1]
            ht_prev = None
            pending_pred = None  # (ht, t) — emitted one step late so the
            # DVE-FIFO predicated copy never blocks the next step's adds
            def flush_pred():
                nonlocal pending_pred
                if pending_pred is None:
                    return
                pht, pt = pending_pred
                slot = pt % HSLOT
                nc.vector.copy_predicated(
                    out=hbig[:, slot * 4 * B:(slot + 1) * 4 * B],
                    mask=maskb[:, pt:pt + 1].to_broadcast([128, 4 * B]),
                    data=pht[:],
                )
                pending_pred = None

            for c in range(NCHUNK):
                xin_cur = xin_q.pop(0)
                emit_g = None
                if c + 2 < NCHUNK:
                    xnew, emit_g = emit_chunk(c + 2)
                    xin_q.append(xnew)
                wc_cast_slice(c)
                for tl in range(CH):
                    t = c * CH + tl
                    ht = rnnp.tile([128, 4 * B], f16, name="ht", tag="ht")
                    ps = psp.tile(
                        [128, 4 * B], f32, space="PSUM",
                        name="psr", tag="psr",
                    )
                    xin3 = xin_cur.rearrange(
                        "p (t m b) -> p t m b", t=CH, m=4, b=B
                    )
                    # one [128,256] identity matmul opens a single
                    # accumulation group for the whole psum tile (sets
                    # has_written everywhere); the 16 W-matmuls then
                    # accumulate into their column slices
                    nc.tensor.matmul(
                        ps[:],
                        lhsT=eye[:],
                        rhs=xin_cur[:, tl * 4 * B:(tl + 1) * 4 * B],
                        start=True,
                        stop=(t == 0),
                        skip_group_check=True,
                    )
                    if t > 0:
                        for m in range(4):
                            for k in range(4):
                                nc.tensor.matmul(
                                    ps[:, m * B:(m + 1) * B],
                                    lhsT=whhT[k][:, m * 128:(m + 1) * 128],
                                    rhs=ht_prev[:, k * B:(k + 1) * B],
                                    start=False,
                                    stop=(m == 3 and k == 3),
                                    skip_group_check=True,
                                )
                    nc.scalar.activation(ht[:], ps[:], AF.Tanh)
                    flush_pred()
                    pending_pred = (ht, t)
                    if emit_g is not None and tl % 2 == 1:
                        emit_g(tl // 2)
                    if t == S - 1:
                        nc.gpsimd.tensor_copy(last_t[:], ht[:])
                    ht_prev = ht
            flush_pred()

            # ------------------------------------------------ attention
            # scores vs last, in hT layout; partition-reduce via ones-matmul.
            # softmax shift = |last|^2 (identical on every core)
            def hdot_step(src, dst, dcol):
                """dst[:, dcol*B:+B] (fp16) = per-(m,b) products of
                <src, last> reduced over m (partial; partitions remain)."""
                mb = attp.tile([128, 4 * B], f16, name="mb", tag="mb", bufs=3)
                nc.vector.tensor_tensor(
                    out=mb[:], in0=src, in1=last_t[:], op=ALU.mult
                )
                with nc.allow_low_precision(reason="4-term fp16 partial sum"):
                    nc.vector.tensor_reduce(
                        out=dst[:, dcol * B:(dcol + 1) * B],
                        in_=mb.rearrange("p (m b) -> p b m", m=4, b=B),
                        op=ALU.add,
                        axis=mybir.AxisListType.X,
                    )

            redL = attp.tile([128, B], f16, name="redL")
            hdot_step(last_t[:], redL, 0)
            psL = psp.tile([1, B], f32, space="PSUM", name="psL", tag="pss")
            nc.tensor.matmul(
                psL[:], lhsT=ones[:], rhs=redL[:], start=True, stop=True
            )
            shf = attp.tile([1, B], f32, name="shf")
            nc.vector.tensor_copy(shf[:], psL[:])

            esc = attp.tile([1, HSLOT * B], f16, name="esc")
            hbig5 = hbig.rearrange(
                "p (q t m b) -> p q t m b", q=8, t=8, m=4, b=B
            )
            lastb = last_t.rearrange(
                "p (one m b) -> p one m b", one=1, m=4, b=B
            )
            for q in range(8):  # 8 t-slices of 8 steps, batched DVE ops
                mbq = attp.tile(
                    [128, 8 * 4 * B], f16, name="mbq", tag="mbq", bufs=2
                )
                nc.vector.tensor_tensor(
                    out=mbq.rearrange("p (t m b) -> p t m b", t=8, m=4, b=B),
                    in0=hbig5[:, q, :, :, :],
                    in1=lastb.to_broadcast([128, 8, 4, B]),
                    op=ALU.mult,
                )
                redb = attp.tile(
                    [128, 8 * B], f16, name="redb", tag="redb", bufs=2
                )
                with nc.allow_low_precision(reason="4-term fp16 partial sum"):
                    nc.vector.tensor_reduce(
                        out=redb.rearrange("p (t b) -> p t b", t=8, b=B),
                        in_=mbq.rearrange(
                            "p (t m b) -> p t b m", t=8, m=4, b=B
                        ),
                        op=ALU.add,
                        axis=mybir.AxisListType.X,
                    )
                pss = psp.tile(
                    [1, 512], f32, space="PSUM", name="pss", tag="pss"
                )
                nc.tensor.matmul(
                    pss[:], lhsT=ones[:], rhs=redb[:], start=True, stop=True
                )
                # exp(score - shift) -> fp16 (bounded: shift ~ max)
                tmp = attp.tile([1, 512], f32, name="tmp", tag="tmp", bufs=2)
                nc.vector.tensor_tensor(
                    out=tmp.rearrange("p (t b) -> p t b", t=8, b=B),
                    in0=pss.rearrange("p (t b) -> p t b", t=8, b=B),
                    in1=shf.rearrange("p (one b) -> p one b", one=1)
                    .to_broadcast([1, 8, B]),
                    op=ALU.subtract,
                )
                nc.scalar.activation(
                    esc[:, q * 512:(q + 1) * 512], tmp[:], AF.Exp
                )
            zf = attp.tile([1, B], f32, name="zf")
            nc.vector.tensor_reduce(
                out=zf[:],
                in_=esc.rearrange("p (t b) -> p b t", t=HSLOT, b=B),
                op=ALU.add,
                axis=mybir.AxisListType.X,
            )
            # ctx accumulation (unnormalized), batched per q-slice
            ctx = attp.tile([128, 4 * B], f32, name="ctx")
            nc.vector.memset(ctx[:], 0.0)
            for q in range(8):
                escb = attp.tile(
                    [128, 8 * B], f16, name="escb", tag="escb", bufs=2
                )
                nc.gpsimd.partition_broadcast(
                    escb[:], esc[:, q * 512:(q + 1) * 512]
                )
                escb4 = escb.rearrange(
                    "p (t one b) -> p t one b", t=8, one=1, b=B
                )
                mcq = attp.tile(
                    [128, 8 * 4 * B], f16, name="mcq", tag="mbq", bufs=2
                )
                nc.vector.tensor_tensor(
                    out=mcq.rearrange("p (t m b) -> p t m b", t=8, m=4, b=B),
                    in0=hbig5[:, q, :, :, :],
                    in1=escb4.to_broadcast([128, 8, 4, B]),
                    op=ALU.mult,
                )
                mct = attp.tile(
                    [128, 4 * B], f32, name="mct", tag="mct", bufs=2
                )
                with nc.allow_low_precision(reason="8-term fp16 partial sum"):
                    nc.vector.tensor_reduce(
                        out=mct[:],
                        in_=mcq.rearrange(
                            "p (t m b) -> p (m b) t", t=8, m=4, b=B
                        ),
                        op=ALU.add,
                        axis=mybir.AxisListType.X,
                    )
                nc.vector.tensor_tensor(
                    out=ctx[:], in0=ctx[:], in1=mct[:], op=ALU.add
                )
            # pack [ctx | z] and AllReduce
            cc_in = drp.tile([128, 4 * B + B], f32, name="cc_in")
            cc_out = drp.tile(
                [128, 4 * B + B], f32, name="cc_out", addr_space="Shared"
            )
            nc.sync.dma_start(cc_in[:, 0:4 * B], ctx[:])
            nc.sync.dma_start(cc_in[0:1, 4 * B:4 * B + B], zf[:])
            nc.gpsimd.collective_compute(
                kind="AllReduce",
                op=ALU.add,
                replica_groups=[list(range(NC))],
                ins=[cc_in[:]],
                outs=[cc_out[:]],
            )
            # final matmul, last-feature half (independent of the
            # collective result -- fills the AllReduce latency bubble);
            # evacuated to SBUF with the output bias folded in
            jn = JC // 8  # 500
            osl = []
            for js in range(8):
                pso = psp.tile(
                    [B, jn], f32, space="PSUM", name="pso", tag="psr"
                )
                for k in range(4, 8):
                    nc.tensor.matmul(
                        pso[:],
                        lhsT=last_t[:, (k % 4) * B:(k % 4 + 1) * B],
                        rhs=wcT[k][:, js * jn:(js + 1) * jn],
                        start=(k == 4),
                        stop=(k == 7),
                    )
                bcs = finp.tile([B, jn], f32, name="bcs", tag="bcs")
                nc.sync.dma_start(bcs[:], bcb_d[:, js * jn:(js + 1) * jn])
                ol = finp.tile([B, jn], f32, name="ol", tag="ol", bufs=8)
                nc.vector.tensor_tensor(
                    out=ol[:], in0=pso[:], in1=bcs[:], op=ALU.add
                )
                osl.append(ol)
            ctxs = attp.tile([128, 4 * B], f32, name="ctxs")
            nc.sync.dma_start(ctxs[:], cc_out[:, 0:4 * B])
            zs = attp.tile([1, B], f32, name="zs")
            nc.sync.dma_start(zs[:], cc_out[0:1, 4 * B:4 * B + B])
            rz = attp.tile([1, B], f32, name="rz")
            nc.vector.reciprocal(rz[:], zs[:])
            rzbs = attp.tile([1, 4 * B], f32, name="rzbs")
            nc.vector.tensor_copy(
                rzbs.rearrange("p (m b) -> p m b", m=4, b=B),
                rz.rearrange("p (one b) -> p one b", one=1).to_broadcast(
                    [1, 4, B]
                ),
            )
            rzb = attp.tile([128, 4 * B], f32, name="rzb")
            nc.gpsimd.partition_broadcast(rzb[:], rzbs[:])
            ctxn = attp.tile([128, 4 * B], f16, name="ctxn")
            nc.vector.tensor_tensor(
                out=ctxn[:], in0=ctxs[:], in1=rzb[:], op=ALU.mult
            )

            # ------------------------------------------------ final matmul
            for js in range(8):
                pso = psp.tile(
                    [B, jn], f32, space="PSUM", name="pso2", tag="psr"
                )
                for k in range(4):
                    nc.tensor.matmul(
                        pso[:],
                        lhsT=ctxn[:, k * B:(k + 1) * B],
                        rhs=wcT[k][:, js * jn:(js + 1) * jn],
                        start=(k == 0),
                        stop=(k == 3),
                    )
                osb = finp.tile([B, jn], f32, name="osb", tag="osb")
                nc.vector.tensor_tensor(
                    out=osb[:], in0=pso[:], in1=osl[js][:], op=ALU.add
                )
                nc.sync.dma_start(out_d[:, js * jn:(js + 1) * jn], osb[:])



    nc.finalize()
    return nc


def _get_nc():
    if "nc" not in _CACHE:
        _CACHE["nc"] = _build()
    return _CACHE["nc"]


def _in_maps(inputs):
    X = np.asarray(inputs["X"], dtype=np.int32)
    emb = np.ascontiguousarray(np.asarray(inputs["emb"], dtype=np.float32))
    W_ih = np.ascontiguousarray(np.asarray(inputs["W_ih"], dtype=np.float32))
    W_hh = np.ascontiguousarray(np.asarray(inputs["W_hh"], dtype=np.float32))
    bias = np.asarray(inputs["b_ih"], dtype=np.float32) + np.asarray(
        inputs["b_hh"], dtype=np.float32
    )
    W = np.asarray(inputs["W"], dtype=np.float32)
    bfull = np.asarray(inputs["b"], dtype=np.float32)
    bias4 = np.ascontiguousarray(bias.reshape(4, 128).T)

    # gather index tile: partition p = two*64+b, free g; token t = 2g+two
    Xr = X.reshape(B, S // 2, 2)                    # [b, g, two]
    xidx = np.ascontiguousarray(
        Xr.transpose(2, 0, 1).reshape(128, S // 2)
    ).astype(np.int32)

    t = np.arange(S)
    maps = []
    for c in range(NC):
        lo, hi = c * HSLOT, (c + 1) * HSLOT
        hmask = ((t >= lo) & (t < hi) & (t != S - 1)).astype(np.float32)
        maps.append(
            {
                "emb": emb,
                "wih": W_ih,
                "whh": W_hh,
                "bias4": bias4,
                "wc": np.ascontiguousarray(W[c * JC:(c + 1) * JC, :]),
                "bcb": np.ascontiguousarray(
                    np.broadcast_to(
                        bfull[c * JC:(c + 1) * JC], (B, JC)
                    ).copy()
                ),
                "xidx": xidx,
                "hmask": hmask.reshape(1, S).astype(np.int16),
                "eye": np.eye(128, dtype=np.float16),
            }
        )
    return maps


def kernel(**inputs) -> np.ndarray:
    from concourse.bass_utils import run_bass_kernel_spmd

    nc = _get_nc()
    res = run_bass_kernel_spmd(
        nc,
        _in_maps(inputs),
        core_ids=list(range(NC)),
        trace=bool(int(os.environ.get("KERNEL_TRACE", "0"))),
    )
    _CACHE["last_results"] = res
    out = np.concatenate([res.results[c]["out"] for c in range(NC)], axis=1)
    return out.astype(np.float32)


if __name__ == "__main__":
    nc = _get_nc()
    print("built OK")
